# revision 28
# baseline (speedup 1.0000x reference)
"""Ragged-sequence attention kernel for 8 Trainium2 NeuronCores.

Problem: N=64 batches, T=2048, D=256.
  energy[n,t] = <key[n,t,:], query[n,:]>
  att = softmax(energy) masked to t < lens[n], renormalized
  context[n,:] = sum_t att[n,t] * value[n,t,:]

Math identity used: global softmax + mask + renormalize == masked softmax
(the global denominator cancels), so we compute exp(e - 20) with an additive
-1e9 mask folded into the energy, then normalize by the masked sum.

Sharding: pure data parallel, batch dim split 8 ways (8 batches/core).
"""

import numpy as np

N, T, D = 64, 2048, 256
NCORES = 8
NB = N // NCORES          # 8 batches per core
P = 128                   # SBUF partitions
CHUNKS = T // P           # 16 chunks of 128 timesteps
HALF = CHUNKS // 2        # key/value loaded in two half-DMAs

_CACHE = {}


def _build():
    import concourse.bass as bass
    import concourse.tile as tile
    from concourse import mybir

    f32 = mybir.dt.float32
    bf16 = mybir.dt.bfloat16
    Alu = mybir.AluOpType

    nc = bass.Bass()

    q_d = nc.declare_dram_parameter("query", [NB, D], f32, isOutput=False)
    k_d = nc.declare_dram_parameter("key", [NB, T, D], f32, isOutput=False)
    # value is uploaded pre-cast to bf16 (host side): halves its HBM read
    v_d = nc.declare_dram_parameter("value", [NB, T, D], bf16, isOutput=False)
    m_d = nc.declare_dram_parameter("addmask", [NB, P, CHUNKS], f32, isOutput=False)
    ctx_d = nc.declare_dram_parameter("ctx_out", [NB, D], f32, isOutput=True)
    att_d = nc.declare_dram_parameter("att_out", [NB, T], f32, isOutput=True)

    # t = p*16 + c -> partition p holds 16 consecutive timesteps; DRAM runs
    # are 8-16 KB contiguous per partition (fast DMA) and the attention
    # output stores directly from [p, c] layout (no transpose needed)
    key4 = k_d[:].rearrange("n (p c) d -> n p c d", c=CHUNKS)
    val4 = v_d[:].rearrange("n (p c) d -> n p c d", c=CHUNKS)
    att3 = att_d[:].rearrange("n (p c) -> n p c", c=CHUNKS)

    with tile.TileContext(nc) as tc:
        with (
            tc.tile_pool(name="const", bufs=1) as constp,
            tc.tile_pool(name="kv", bufs=5) as kvp,
            tc.tile_pool(name="small", bufs=3) as smp,
            tc.tile_pool(name="scratch", bufs=2) as scp,
            tc.tile_pool(name="ps", bufs=8, space="PSUM") as psp,
        ):
            ones_col = constp.tile([P, 1], f32)
            nc.vector.memset(ones_col[:], 1.0)
            ones_row = constp.tile([1, P], f32)
            nc.vector.memset(ones_row[:], 1.0)
            bias_sb = constp.tile([P, 1], f32)
            nc.vector.memset(bias_sb[:], -20.0)

            for n in range(NB):
                # q broadcast across partitions via stride-0 DMA
                q_rep = smp.tile([P, D], f32, tag="q_rep")
                nc.sync.dma_start(q_rep[:], q_d[n : n + 1, :].to_broadcast((P, D)))
                mask_sb = smp.tile([P, CHUNKS], f32, tag="mask")
                nc.sync.dma_start(mask_sb[:], m_d[n])

                kts, vts = [], []
                for h in range(2):
                    kt = kvp.tile([P, HALF, D], f32, tag=f"kt{h}", bufs=6)
                    nc.sync.dma_start(kt[:], key4[n, :, h * HALF : (h + 1) * HALF, :])
                    kts.append(kt)
                for h in range(2):
                    vt = kvp.tile([P, HALF, D], bf16, tag=f"vt{h}", bufs=6)
                    nc.sync.dma_start(vt[:], val4[n, :, h * HALF : (h + 1) * HALF, :])
                    vts.append(vt)

                # energy e[p, c] = sum_d key[p, c, d] * q[d]
                # DVE: broadcast multiply per half; reduction split between
                # DVE (one 3D reduce for half 0) and ACT (copy+accum, half 1)
                e_sb = smp.tile([P, CHUNKS], f32, tag="e_sb")
                q_bcast = q_rep[:][:, None, :].to_broadcast((P, HALF, D))
                prods = []
                for h in range(2):
                    prod = scp.tile([P, HALF, D], f32, tag=f"scr{h}")
                    nc.vector.tensor_tensor(prod[:], kts[h][:], q_bcast, Alu.mult)
                    prods.append(prod)
                nc.vector.tensor_reduce(
                    e_sb[:, 0:HALF], prods[0][:], mybir.AxisListType.X, Alu.add
                )
                for c in range(HALF):
                    trash = scp.tile([P, D], f32, tag="trash", bufs=1)
                    nc.scalar.activation(
                        trash[:],
                        prods[1][:, c, :],
                        mybir.ActivationFunctionType.Copy,
                        accum_out=e_sb[:, HALF + c : HALF + c + 1],
                    )

                # additive mask (-1e9 at t >= lens), then exp with fused row-sum
                em_sb = smp.tile([P, CHUNKS], f32, tag="em_sb")
                nc.vector.tensor_tensor(em_sb[:], e_sb[:], mask_sb[:], Alu.add)
                p_sb = smp.tile([P, CHUNKS], f32, tag="p_sb")
                part_sb = smp.tile([P, 1], f32, tag="part")
                nc.scalar.activation(
                    p_sb[:],
                    em_sb[:],
                    mybir.ActivationFunctionType.Exp,
                    bias=bias_sb[:],
                    scale=1.0,
                    accum_out=part_sb[:],
                )

                # total s = sum over partitions of part, r = 1/s
                s_ps = psp.tile([1, 1], f32, tag="ps")
                nc.tensor.matmul(s_ps[:], ones_col[:], part_sb[:])
                r_sb = smp.tile([1, 1], f32, tag="r_sb")
                nc.vector.reciprocal(r_sb[:], s_ps[:])

                # context[d] = r * sum_t p[t] * value[t, d] (accumulate over chunks)
                # bf16 weights/values -> single-pass matmuls on PE
                p_bf = smp.tile([P, CHUNKS], bf16, tag="p_bf")
                nc.vector.tensor_copy(p_bf[:], p_sb[:])
                ctx_ps = psp.tile([1, D], f32, tag="ps")
                for c in range(CHUNKS):
                    nc.tensor.matmul(
                        ctx_ps[:],
                        p_bf[:, c : c + 1],
                        vts[c // HALF][:, c % HALF, :],
                        start=(c == 0),
                        stop=(c == CHUNKS - 1),
                    )
                ctx_sb = smp.tile([1, D], f32, tag="ctx_sb")
                nc.scalar.mul(ctx_sb[:], ctx_ps[:], mul=r_sb[:])
                nc.scalar.dma_start(ctx_d[n : n + 1, :], ctx_sb[:])

                # attention out: att[p, c] = p[p, c] * r, stored directly
                rb_ps = psp.tile([P, 1], f32, tag="ps")
                nc.tensor.matmul(rb_ps[:], ones_row[:], r_sb[:])
                rb_sb = smp.tile([P, 1], f32, tag="rb_sb")
                nc.vector.tensor_copy(rb_sb[:], rb_ps[:])
                att_sb = smp.tile([P, CHUNKS], f32, tag="att_sb")
                nc.scalar.mul(att_sb[:], p_sb[:], mul=rb_sb[:])
                nc.scalar.dma_start(att3[n], att_sb[:])

    return nc


def _split_multiwaits(bir):
    """The walrus build in this container allows only ONE sync wait per
    instruction (setupSyncWait: 'Too many sync wait commands'). Tile attaches
    multiple waits to single instructions. Split the extras into standalone
    EventSemaphore wait instructions (same engine, placed immediately before)
    — semantically identical, just sequential waits."""
    ctr = 0
    for fn in bir["functions"]:
        for blk in fn["blocks"]:
            out = []
            for inst in blk["instructions"]:
                si = inst.get("sync_info")
                waits = (si or {}).get("on_wait") or []
                # raw-bytes ISA instructions encode waits in their payload;
                # rewriting sync_info desyncs the encoded length
                if len(waits) > 1 and not inst.get("instr"):
                    for w in waits[:-1]:
                        ctr += 1
                        pre = {
                            "name": f"I-mw{ctr}",
                            "opcode": "EventSemaphore",
                            "engine": inst["engine"],
                            "ins": [],
                            "outs": [],
                            "sync_info": {"on_update": [], "on_wait": [w]},
                        }
                        if "debug" in inst:
                            pre["debug"] = inst["debug"]
                        out.append(pre)
                    si["on_wait"] = [waits[-1]]
                out.append(inst)
            blk["instructions"] = out
    return bir


def _patch_json(nc):
    import json as _json

    orig = nc.to_json_bytes

    def patched():
        bir = _json.loads(orig())
        _split_multiwaits(bir)
        return _json.dumps(bir).encode()

    nc.to_json_bytes = patched
    return nc


def _get_nc():
    if "nc" not in _CACHE:
        _CACHE["nc"] = _patch_json(_build())
    return _CACHE["nc"]


def _make_in_maps(query, key, value, lens):
    import ml_dtypes

    query = np.ascontiguousarray(np.asarray(query, dtype=np.float32))
    key = np.ascontiguousarray(np.asarray(key, dtype=np.float32))
    value = np.ascontiguousarray(
        np.asarray(value, dtype=np.float32).astype(ml_dtypes.bfloat16)
    )
    lens = np.asarray(lens).astype(np.int64)

    # addmask[n, p, c] = 0 if p*16+c < lens[n] else -1e9
    t_idx = np.arange(T).reshape(P, CHUNKS)
    addmask = np.where(t_idx[None, :, :] < lens[:, None, None], 0.0, -1e9).astype(
        np.float32
    )

    in_maps = []
    for i in range(NCORES):
        sl = slice(i * NB, (i + 1) * NB)
        in_maps.append(
            {
                "query": query[sl],
                "key": key[sl],
                "value": value[sl],
                "addmask": np.ascontiguousarray(addmask[sl]),
            }
        )
    return in_maps


def _run(in_maps, trace=False, **kwargs):
    from concourse.bass_utils import run_bass_kernel_spmd

    nc = _get_nc()
    return run_bass_kernel_spmd(
        nc, in_maps, core_ids=list(range(NCORES)), trace=trace, **kwargs
    )


def kernel(query, key, value, lens):
    in_maps = _make_in_maps(query, key, value, lens)
    res = _run(in_maps, trace=False)
    ctx = np.concatenate([r["ctx_out"] for r in res.results], axis=0)
    att = np.concatenate([r["att_out"] for r in res.results], axis=0)
    return ctx, att


# revision 36
# speedup vs baseline: 1.1610x; 1.1610x over previous
"""Ragged-sequence attention kernel for 8 Trainium2 NeuronCores.

Problem: N=64 batches, T=2048, D=256.
  energy[n,t] = <key[n,t,:], query[n,:]>
  att = softmax(energy) masked to t < lens[n], renormalized
  context[n,:] = sum_t att[n,t] * value[n,t,:]

Math identity used: global softmax + mask + renormalize == masked softmax
(the global denominator cancels), so we compute p = exp(e - 20 + addmask)
with addmask in {0, -1e9}, accumulate UNNORMALIZED context = sum_t p*v, and
apply r = 1/sum_t(p) to both outputs at the very end.

Sharding: pure data parallel, batch dim split 8 ways (8 batches/core).

Device structure (v8):
  - energy on PE: host-transposed keyT[n] = key[n].T so e = qT.T @ keyT runs
    as N=512 matmuls; energy lands [8 batches x T] with batches in
    partitions, making the whole softmax a handful of 8-lane ops.
  - context on PE in bf16 (value uploaded pre-cast + pre-permuted), with
    per-128-chunk PE transposes of p to put t into partitions.
  - t processed in 4 groups of 512, software-pipelined: group g's energy
    matmuls run on PE while group g-1's softmax (DVE/ACT) completes, so PE
    never waits on the softmax chain.
"""

import numpy as np

N, T, D = 64, 2048, 256
NCORES = 8
NB = N // NCORES          # 8 batches per core
P = 128
GROUPS = 4
GT = T // GROUPS          # 512 timesteps per group
GC = GT // P              # 4 chunks of 128 t per group
NCHUNKS = T // P          # 16 chunks total
DH = D // P               # 2 halves of the d contraction

_CACHE = {}


def _build():
    import concourse.bass as bass
    import concourse.tile as tile
    from concourse import mybir

    f32 = mybir.dt.float32
    bf16 = mybir.dt.bfloat16
    Alu = mybir.AluOpType

    nc = bass.Bass()

    # qTm[p, dh, n, col] = query[n, dh*128+p] if col==n else 0 — masked weight
    # slab so each energy matmul writes the full [NB, GT] PSUM tile (PE output
    # base partition must be 0/32/64, so per-row outputs are not allowed)
    qt_d = nc.declare_dram_parameter("qTm", [P, DH, NB, NB], f32, isOutput=False)
    # keyT[n] = key[n].T  (host-transposed, f32)
    kt_d = nc.declare_dram_parameter("keyT", [NB, D, T], f32, isOutput=False)
    # valP[n, g, p, c, d] = value[n, g*512 + c*128 + p, d]  (bf16)
    v_d = nc.declare_dram_parameter(
        "valP", [NB, GROUPS, P, GC, D], bf16, isOutput=False
    )
    m_d = nc.declare_dram_parameter("addmask", [NB, T], f32, isOutput=False)
    id_d = nc.declare_dram_parameter("ident8", [NB, NB], f32, isOutput=False)
    eye_d = nc.declare_dram_parameter("eye8bf", [1, NB, NB], bf16, isOutput=False)
    ctx_d = nc.declare_dram_parameter("ctx_out", [NB, D], f32, isOutput=True)
    att_d = nc.declare_dram_parameter("att_out", [NB, T], f32, isOutput=True)

    # keyT tile view: [n, dh, p(d), g, t-in-group]
    key5 = kt_d[:].rearrange("n (dh p) (g t) -> n dh p g t", p=P, g=GROUPS)

    with tile.TileContext(nc) as tc:
        with (
            tc.tile_pool(name="const", bufs=1) as constp,
            tc.tile_pool(name="kv", bufs=1) as kvp,
            tc.tile_pool(name="big", bufs=1) as bigp,
            tc.tile_pool(name="small", bufs=2) as smp,
            tc.tile_pool(name="pst", bufs=2, space="PSUM") as pstp,
            tc.tile_pool(name="pse", bufs=2, space="PSUM") as psep,
            tc.tile_pool(name="psc", bufs=1, space="PSUM") as pscp,
        ):
            qt_sb = constp.tile([P, DH, NB, NB], f32)
            nc.sync.dma_start(qt_sb[:], qt_d[:])
            ident = constp.tile([NB, NB], f32)
            nc.sync.dma_start(ident[:], id_d[:])
            # eye8 replicated across partitions: eyeRep[p, n, col] = (col==n)
            eye_rep = constp.tile([P, NB, NB], bf16)
            nc.sync.dma_start(eye_rep[:], eye_d[:].to_broadcast((P, NB, NB)))
            bias_sb = constp.tile([NB, 1], f32)
            nc.vector.memset(bias_sb[:], -20.0)
            mask_sb = bigp.tile([NB, T], f32)
            nc.sync.dma_start(mask_sb[:], m_d[:])

            p_sb = bigp.tile([NB, T], f32)
            parts = bigp.tile([NB, GROUPS], f32)
            pT_sb = bigp.tile([P, NCHUNKS, NB], bf16)
            ctx_ps = pscp.tile([NB, D], f32)

            e_tiles = {}
            v_tiles = {}

            def emit_energy(g):
                e_ps = psep.tile([NB, GT], f32, tag="eps")
                e_tiles[g] = e_ps
                for n in range(NB):
                    kts = []
                    for dh in range(DH):
                        kt = kvp.tile([P, GT], f32, tag="kt", bufs=32)
                        nc.sync.dma_start(kt[:], key5[n, dh, :, g, :])
                        kts.append(kt)
                    for dh in range(DH):
                        nc.tensor.matmul(
                            e_ps[:],
                            qt_sb[:, dh, n, :],
                            kts[dh][:],
                            start=(n == 0 and dh == 0),
                            stop=(n == NB - 1 and dh == DH - 1),
                        )
                    vt = kvp.tile([P, GC, D], bf16, tag="vt", bufs=16)
                    nc.sync.dma_start(vt[:], v_d[n, g])
                    v_tiles[(n, g)] = vt

            def emit_posts(g):
                sl = slice(g * GT, (g + 1) * GT)
                em = smp.tile([NB, GT], f32, tag="em")
                nc.vector.tensor_tensor(
                    em[:], e_tiles[g][:], mask_sb[:, sl], Alu.add
                )
                nc.scalar.activation(
                    p_sb[:, sl],
                    em[:],
                    mybir.ActivationFunctionType.Exp,
                    bias=bias_sb[:],
                    accum_out=parts[:, g : g + 1],
                )
                for c in range(GC):
                    cg = g * GC + c
                    pT_ps = pstp.tile([P, NB], f32, tag="pt")
                    nc.tensor.transpose(
                        pT_ps[:], p_sb[:, cg * P : (cg + 1) * P], ident[:]
                    )
                    nc.vector.tensor_copy(pT_sb[:, cg, :], pT_ps[:])
                for n in range(NB):
                    vt = v_tiles.pop((n, g))
                    for c in range(GC):
                        cg = g * GC + c
                        # masked slab: only column n of p survives, so the
                        # matmul writes ctx row n and zeros elsewhere
                        pTm = smp.tile([P, NB], bf16, tag="ptm", bufs=16)
                        nc.vector.tensor_tensor(
                            pTm[:], pT_sb[:, cg, :], eye_rep[:, n, :], Alu.mult
                        )
                        nc.tensor.matmul(
                            ctx_ps[:],
                            pTm[:],
                            vt[:, c, :],
                            start=(g == 0 and n == 0 and c == 0),
                            stop=(
                                g == GROUPS - 1 and n == NB - 1 and c == GC - 1
                            ),
                        )

            for g in range(GROUPS):
                emit_energy(g)
                if g >= 1:
                    emit_posts(g - 1)
            emit_posts(GROUPS - 1)

            # tail: s, r, normalize both outputs
            s_sb = smp.tile([NB, 1], f32, tag="s")
            nc.vector.tensor_reduce(s_sb[:], parts[:], mybir.AxisListType.X, Alu.add)
            r_sb = smp.tile([NB, 1], f32, tag="r")
            nc.vector.reciprocal(r_sb[:], s_sb[:])
            att_sb = bigp.tile([NB, T], f32)
            nc.vector.tensor_scalar_mul(att_sb[:], p_sb[:], r_sb[:])
            nc.scalar.dma_start(att_d[:], att_sb[:])
            ctx_sb = smp.tile([NB, D], f32, tag="ctx")
            nc.scalar.mul(ctx_sb[:], ctx_ps[:], mul=r_sb[:])
            nc.scalar.dma_start(ctx_d[:], ctx_sb[:])

    return nc


def _split_multiwaits(bir):
    """The walrus build in this container allows only ONE sync wait per
    instruction (setupSyncWait: 'Too many sync wait commands'). Tile attaches
    multiple waits to single instructions. Split the extras into standalone
    EventSemaphore wait instructions (same engine, placed immediately before)
    — semantically identical, just sequential waits."""
    ctr = 0
    for fn in bir["functions"]:
        for blk in fn["blocks"]:
            out = []
            for inst in blk["instructions"]:
                si = inst.get("sync_info")
                waits = (si or {}).get("on_wait") or []
                # raw-bytes ISA instructions encode waits in their payload;
                # rewriting sync_info desyncs the encoded length
                if len(waits) > 1 and not inst.get("instr"):
                    for w in waits[:-1]:
                        ctr += 1
                        pre = {
                            "name": f"I-mw{ctr}",
                            "opcode": "EventSemaphore",
                            "engine": inst["engine"],
                            "ins": [],
                            "outs": [],
                            "sync_info": {"on_update": [], "on_wait": [w]},
                        }
                        if "debug" in inst:
                            pre["debug"] = inst["debug"]
                        out.append(pre)
                    si["on_wait"] = [waits[-1]]
                out.append(inst)
            blk["instructions"] = out
    return bir


def _patch_json(nc):
    import json as _json

    orig = nc.to_json_bytes

    def patched():
        bir = _json.loads(orig())
        _split_multiwaits(bir)
        return _json.dumps(bir).encode()

    nc.to_json_bytes = patched
    return nc


def _get_nc():
    if "nc" not in _CACHE:
        _CACHE["nc"] = _patch_json(_build())
    return _CACHE["nc"]


def _make_in_maps(query, key, value, lens):
    import ml_dtypes

    query = np.asarray(query, dtype=np.float32)
    key = np.asarray(key, dtype=np.float32)
    value = np.asarray(value, dtype=np.float32)
    lens = np.asarray(lens).astype(np.int64)

    addmask = np.where(
        np.arange(T)[None, :] < lens[:, None], 0.0, -1e9
    ).astype(np.float32)
    ident8 = np.eye(NB, dtype=np.float32)
    eye8bf = np.eye(NB, dtype=ml_dtypes.bfloat16)[None]

    in_maps = []
    for i in range(NCORES):
        sl = slice(i * NB, (i + 1) * NB)
        q = query[sl]  # (NB, D)
        # qT[p, dh, n] = q[n, dh*128+p]; masked slab qTm[..., n, col!=n] = 0
        qT = q.T.reshape(DH, P, NB).transpose(1, 0, 2)
        qTm = np.zeros((P, DH, NB, NB), dtype=np.float32)
        qTm[:, :, np.arange(NB), np.arange(NB)] = qT
        keyT = np.ascontiguousarray(key[sl].transpose(0, 2, 1))
        valP = np.ascontiguousarray(
            value[sl]
            .reshape(NB, GROUPS, GC, P, D)
            .transpose(0, 1, 3, 2, 4)
            .astype(ml_dtypes.bfloat16)
        )
        in_maps.append(
            {
                "qTm": qTm,
                "keyT": keyT,
                "valP": valP,
                "addmask": np.ascontiguousarray(addmask[sl]),
                "ident8": ident8,
                "eye8bf": eye8bf,
            }
        )
    return in_maps


def _run(in_maps, trace=False, **kwargs):
    from concourse.bass_utils import run_bass_kernel_spmd

    nc = _get_nc()
    return run_bass_kernel_spmd(
        nc, in_maps, core_ids=list(range(NCORES)), trace=trace, **kwargs
    )


def kernel(query, key, value, lens):
    in_maps = _make_in_maps(query, key, value, lens)
    res = _run(in_maps, trace=False)
    ctx = np.concatenate([r["ctx_out"] for r in res.results], axis=0)
    att = np.concatenate([r["att_out"] for r in res.results], axis=0)
    return ctx, att


# revision 38
# speedup vs baseline: 1.2979x; 1.1179x over previous
"""Ragged-sequence attention kernel for 8 Trainium2 NeuronCores.

Problem: N=64 batches, T=2048, D=256.
  energy[n,t] = <key[n,t,:], query[n,:]>
  att = softmax(energy) masked to t < lens[n], renormalized
  context[n,:] = sum_t att[n,t] * value[n,t,:]

Math identity used: global softmax + mask + renormalize == masked softmax
(the global denominator cancels), so we compute p = exp(e - 20 + addmask)
with addmask in {0, -1e9}, accumulate UNNORMALIZED context = sum_t p*v, and
apply r = 1/sum_t(p) to both outputs at the very end.

Sharding: pure data parallel, batch dim split 8 ways (8 batches/core).

Device structure (v8):
  - energy on PE: host-transposed keyT[n] = key[n].T so e = qT.T @ keyT runs
    as N=512 matmuls; energy lands [8 batches x T] with batches in
    partitions, making the whole softmax a handful of 8-lane ops.
  - context on PE in bf16 (value uploaded pre-cast + pre-permuted), with
    per-128-chunk PE transposes of p to put t into partitions.
  - t processed in 4 groups of 512, software-pipelined: group g's energy
    matmuls run on PE while group g-1's softmax (DVE/ACT) completes, so PE
    never waits on the softmax chain.
"""

import numpy as np

N, T, D = 64, 2048, 256
NCORES = 8
NB = N // NCORES          # 8 batches per core
P = 128
GROUPS = 4
GT = T // GROUPS          # 512 timesteps per group
GC = GT // P              # 4 chunks of 128 t per group
NCHUNKS = T // P          # 16 chunks total
DH = D // P               # 2 halves of the d contraction

_CACHE = {}


def _build():
    import concourse.bass as bass
    import concourse.tile as tile
    from concourse import mybir

    f32 = mybir.dt.float32
    bf16 = mybir.dt.bfloat16
    f16 = mybir.dt.float16
    Alu = mybir.AluOpType

    nc = bass.Bass()

    # qTm[p, dh, n, col] = query[n, dh*128+p] if col==n else 0 — masked weight
    # slab so each energy matmul writes the full [NB, GT] PSUM tile (PE output
    # base partition must be 0/32/64, so per-row outputs are not allowed)
    qt_d = nc.declare_dram_parameter("qTm", [P, DH, NB, NB], f16, isOutput=False)
    # keyT[n] = key[n].T  (host-transposed, f32)
    kt_d = nc.declare_dram_parameter("keyT", [NB, D, T], f16, isOutput=False)
    # valP[n, g, p, c, d] = value[n, g*512 + c*128 + p, d]  (bf16)
    v_d = nc.declare_dram_parameter(
        "valP", [NB, GROUPS, P, GC, D], bf16, isOutput=False
    )
    m_d = nc.declare_dram_parameter("addmask", [NB, T], f32, isOutput=False)
    id_d = nc.declare_dram_parameter("ident8", [NB, NB], f32, isOutput=False)
    eye_d = nc.declare_dram_parameter("eye8bf", [1, NB, NB], bf16, isOutput=False)
    ctx_d = nc.declare_dram_parameter("ctx_out", [NB, D], f32, isOutput=True)
    att_d = nc.declare_dram_parameter("att_out", [NB, T], f32, isOutput=True)

    # keyT tile view: [n, dh, p(d), g, t-in-group]
    key5 = kt_d[:].rearrange("n (dh p) (g t) -> n dh p g t", p=P, g=GROUPS)

    with tile.TileContext(nc) as tc:
        with (
            tc.tile_pool(name="const", bufs=1) as constp,
            tc.tile_pool(name="kv", bufs=1) as kvp,
            tc.tile_pool(name="big", bufs=1) as bigp,
            tc.tile_pool(name="small", bufs=2) as smp,
            tc.tile_pool(name="pst", bufs=2, space="PSUM") as pstp,
            tc.tile_pool(name="pse", bufs=2, space="PSUM") as psep,
            tc.tile_pool(name="psc", bufs=1, space="PSUM") as pscp,
        ):
            qt_sb = constp.tile([P, DH, NB, NB], f16)
            nc.sync.dma_start(qt_sb[:], qt_d[:])
            ident = constp.tile([NB, NB], f32)
            nc.sync.dma_start(ident[:], id_d[:])
            # eye8 replicated across partitions: eyeRep[p, n, col] = (col==n)
            eye_rep = constp.tile([P, NB, NB], bf16)
            nc.sync.dma_start(eye_rep[:], eye_d[:].to_broadcast((P, NB, NB)))
            bias_sb = constp.tile([NB, 1], f32)
            nc.vector.memset(bias_sb[:], -20.0)
            mask_sb = bigp.tile([NB, T], f32)
            nc.sync.dma_start(mask_sb[:], m_d[:])

            p_sb = bigp.tile([NB, T], f32)
            parts = bigp.tile([NB, GROUPS], f32)
            pT_sb = bigp.tile([P, NCHUNKS, NB], bf16)
            ctx_ps = pscp.tile([NB, D], f32)

            e_tiles = {}
            v_tiles = {}

            def emit_energy(g):
                e_ps = psep.tile([NB, GT], f32, tag="eps")
                e_tiles[g] = e_ps
                for n in range(NB):
                    kts = []
                    for dh in range(DH):
                        kt = kvp.tile([P, GT], f16, tag="kt", bufs=32)
                        nc.sync.dma_start(kt[:], key5[n, dh, :, g, :])
                        kts.append(kt)
                    for dh in range(DH):
                        nc.tensor.matmul(
                            e_ps[:],
                            qt_sb[:, dh, n, :],
                            kts[dh][:],
                            start=(n == 0 and dh == 0),
                            stop=(n == NB - 1 and dh == DH - 1),
                        )
                    vt = kvp.tile([P, GC, D], bf16, tag="vt", bufs=16)
                    nc.sync.dma_start(vt[:], v_d[n, g])
                    v_tiles[(n, g)] = vt

            def emit_posts(g):
                sl = slice(g * GT, (g + 1) * GT)
                em = smp.tile([NB, GT], f32, tag="em")
                nc.vector.tensor_tensor(
                    em[:], e_tiles[g][:], mask_sb[:, sl], Alu.add
                )
                nc.scalar.activation(
                    p_sb[:, sl],
                    em[:],
                    mybir.ActivationFunctionType.Exp,
                    bias=bias_sb[:],
                    accum_out=parts[:, g : g + 1],
                )
                for c in range(GC):
                    cg = g * GC + c
                    pT_ps = pstp.tile([P, NB], f32, tag="pt")
                    nc.tensor.transpose(
                        pT_ps[:], p_sb[:, cg * P : (cg + 1) * P], ident[:]
                    )
                    nc.vector.tensor_copy(pT_sb[:, cg, :], pT_ps[:])
                for n in range(NB):
                    vt = v_tiles.pop((n, g))
                    for c in range(GC):
                        cg = g * GC + c
                        # masked slab: only column n of p survives, so the
                        # matmul writes ctx row n and zeros elsewhere
                        pTm = smp.tile([P, NB], bf16, tag="ptm", bufs=16)
                        nc.vector.tensor_tensor(
                            pTm[:], pT_sb[:, cg, :], eye_rep[:, n, :], Alu.mult
                        )
                        nc.tensor.matmul(
                            ctx_ps[:],
                            pTm[:],
                            vt[:, c, :],
                            start=(g == 0 and n == 0 and c == 0),
                            stop=(
                                g == GROUPS - 1 and n == NB - 1 and c == GC - 1
                            ),
                        )

            for g in range(GROUPS):
                emit_energy(g)
                if g >= 1:
                    emit_posts(g - 1)
            emit_posts(GROUPS - 1)

            # tail: s, r, normalize both outputs
            s_sb = smp.tile([NB, 1], f32, tag="s")
            nc.vector.tensor_reduce(s_sb[:], parts[:], mybir.AxisListType.X, Alu.add)
            r_sb = smp.tile([NB, 1], f32, tag="r")
            nc.vector.reciprocal(r_sb[:], s_sb[:])
            att_sb = bigp.tile([NB, T], f32)
            nc.vector.tensor_scalar_mul(att_sb[:], p_sb[:], r_sb[:])
            nc.scalar.dma_start(att_d[:], att_sb[:])
            ctx_sb = smp.tile([NB, D], f32, tag="ctx")
            nc.scalar.mul(ctx_sb[:], ctx_ps[:], mul=r_sb[:])
            nc.scalar.dma_start(ctx_d[:], ctx_sb[:])

    return nc


def _split_multiwaits(bir):
    """The walrus build in this container allows only ONE sync wait per
    instruction (setupSyncWait: 'Too many sync wait commands'). Tile attaches
    multiple waits to single instructions. Split the extras into standalone
    EventSemaphore wait instructions (same engine, placed immediately before)
    — semantically identical, just sequential waits."""
    ctr = 0
    for fn in bir["functions"]:
        for blk in fn["blocks"]:
            out = []
            for inst in blk["instructions"]:
                si = inst.get("sync_info")
                waits = (si or {}).get("on_wait") or []
                # raw-bytes ISA instructions encode waits in their payload;
                # rewriting sync_info desyncs the encoded length
                if len(waits) > 1 and not inst.get("instr"):
                    for w in waits[:-1]:
                        ctr += 1
                        pre = {
                            "name": f"I-mw{ctr}",
                            "opcode": "EventSemaphore",
                            "engine": inst["engine"],
                            "ins": [],
                            "outs": [],
                            "sync_info": {"on_update": [], "on_wait": [w]},
                        }
                        if "debug" in inst:
                            pre["debug"] = inst["debug"]
                        out.append(pre)
                    si["on_wait"] = [waits[-1]]
                out.append(inst)
            blk["instructions"] = out
    return bir


def _patch_json(nc):
    import json as _json

    orig = nc.to_json_bytes

    def patched():
        bir = _json.loads(orig())
        _split_multiwaits(bir)
        return _json.dumps(bir).encode()

    nc.to_json_bytes = patched
    return nc


def _get_nc():
    if "nc" not in _CACHE:
        _CACHE["nc"] = _patch_json(_build())
    return _CACHE["nc"]


def _make_in_maps(query, key, value, lens):
    import ml_dtypes

    query = np.asarray(query, dtype=np.float32)
    key = np.asarray(key, dtype=np.float32)
    value = np.asarray(value, dtype=np.float32)
    lens = np.asarray(lens).astype(np.int64)

    addmask = np.where(
        np.arange(T)[None, :] < lens[:, None], 0.0, -1e9
    ).astype(np.float32)
    ident8 = np.eye(NB, dtype=np.float32)
    eye8bf = np.eye(NB, dtype=ml_dtypes.bfloat16)[None]

    in_maps = []
    for i in range(NCORES):
        sl = slice(i * NB, (i + 1) * NB)
        q = query[sl]  # (NB, D)
        # qT[p, dh, n] = q[n, dh*128+p]; masked slab qTm[..., n, col!=n] = 0
        qT = q.T.reshape(DH, P, NB).transpose(1, 0, 2)
        qTm = np.zeros((P, DH, NB, NB), dtype=np.float16)
        qTm[:, :, np.arange(NB), np.arange(NB)] = qT.astype(np.float16)
        keyT = np.ascontiguousarray(key[sl].transpose(0, 2, 1).astype(np.float16))
        valP = np.ascontiguousarray(
            value[sl]
            .reshape(NB, GROUPS, GC, P, D)
            .transpose(0, 1, 3, 2, 4)
            .astype(ml_dtypes.bfloat16)
        )
        in_maps.append(
            {
                "qTm": qTm,
                "keyT": keyT,
                "valP": valP,
                "addmask": np.ascontiguousarray(addmask[sl]),
                "ident8": ident8,
                "eye8bf": eye8bf,
            }
        )
    return in_maps


def _run(in_maps, trace=False, **kwargs):
    from concourse.bass_utils import run_bass_kernel_spmd

    nc = _get_nc()
    return run_bass_kernel_spmd(
        nc, in_maps, core_ids=list(range(NCORES)), trace=trace, **kwargs
    )


def kernel(query, key, value, lens):
    in_maps = _make_in_maps(query, key, value, lens)
    res = _run(in_maps, trace=False)
    ctx = np.concatenate([r["ctx_out"] for r in res.results], axis=0)
    att = np.concatenate([r["att_out"] for r in res.results], axis=0)
    return ctx, att


# revision 40
# speedup vs baseline: 1.3325x; 1.0267x over previous
"""Ragged-sequence attention kernel for 8 Trainium2 NeuronCores.

Problem: N=64 batches, T=2048, D=256.
  energy[n,t] = <key[n,t,:], query[n,:]>
  att = softmax(energy) masked to t < lens[n], renormalized
  context[n,:] = sum_t att[n,t] * value[n,t,:]

Math identity used: global softmax + mask + renormalize == masked softmax
(the global denominator cancels), so we compute p = exp(e - 20 + addmask)
with addmask in {0, -1e9}, accumulate UNNORMALIZED context = sum_t p*v, and
apply r = 1/sum_t(p) to both outputs at the very end.

Sharding: pure data parallel, batch dim split 8 ways (8 batches/core).

Device structure (v8):
  - energy on PE: host-transposed keyT[n] = key[n].T so e = qT.T @ keyT runs
    as N=512 matmuls; energy lands [8 batches x T] with batches in
    partitions, making the whole softmax a handful of 8-lane ops.
  - context on PE in bf16 (value uploaded pre-cast + pre-permuted), with
    per-128-chunk PE transposes of p to put t into partitions.
  - t processed in 4 groups of 512, software-pipelined: group g's energy
    matmuls run on PE while group g-1's softmax (DVE/ACT) completes, so PE
    never waits on the softmax chain.
"""

import numpy as np

N, T, D = 64, 2048, 256
NCORES = 8
NB = N // NCORES          # 8 batches per core
P = 128
GROUPS = 4
GT = T // GROUPS          # 512 timesteps per group
GC = GT // P              # 4 chunks of 128 t per group
NCHUNKS = T // P          # 16 chunks total
DH = D // P               # 2 halves of the d contraction

_CACHE = {}


def _build():
    import concourse.bass as bass
    import concourse.tile as tile
    from concourse import mybir

    f32 = mybir.dt.float32
    bf16 = mybir.dt.bfloat16
    f16 = mybir.dt.float16
    Alu = mybir.AluOpType

    nc = bass.Bass()

    # qTm[p, dh, n, col] = query[n, dh*128+p] if col==n else 0 — masked weight
    # slab so each energy matmul writes the full [NB, GT] PSUM tile (PE output
    # base partition must be 0/32/64, so per-row outputs are not allowed)
    qt_d = nc.declare_dram_parameter("qTm", [P, DH, NB, NB], f16, isOutput=False)
    # keyT[n] = key[n].T  (host-transposed, f32)
    # kth[n, p, dh, t] = key[n, t, dh*128+p] — 8 KB contiguous per partition
    kt_d = nc.declare_dram_parameter("keyT", [NB, P, DH, T], f16, isOutput=False)
    # valP[n, g, p, c, d] = value[n, g*512 + c*128 + p, d]  (bf16)
    # valP[n, p, cg, d] = value[n, (cg//4)*512 + (cg%4)*128 + p, d]
    v_d = nc.declare_dram_parameter("valP", [NB, P, NCHUNKS, D], bf16, isOutput=False)
    m_d = nc.declare_dram_parameter("addmask", [NB, T], f32, isOutput=False)
    id_d = nc.declare_dram_parameter("ident8", [NB, NB], f32, isOutput=False)
    eye_d = nc.declare_dram_parameter("eye8bf", [1, NB, NB], bf16, isOutput=False)
    ctx_d = nc.declare_dram_parameter("ctx_out", [NB, D], f32, isOutput=True)
    att_d = nc.declare_dram_parameter("att_out", [NB, T], f32, isOutput=True)


    with tile.TileContext(nc) as tc:
        with (
            tc.tile_pool(name="const", bufs=1) as constp,
            tc.tile_pool(name="kv", bufs=1) as kvp,
            tc.tile_pool(name="big", bufs=1) as bigp,
            tc.tile_pool(name="small", bufs=2) as smp,
            tc.tile_pool(name="pst", bufs=2, space="PSUM") as pstp,
            tc.tile_pool(name="pse", bufs=2, space="PSUM") as psep,
            tc.tile_pool(name="psc", bufs=1, space="PSUM") as pscp,
        ):
            qt_sb = constp.tile([P, DH, NB, NB], f16)
            nc.sync.dma_start(qt_sb[:], qt_d[:])
            ident = constp.tile([NB, NB], f32)
            nc.sync.dma_start(ident[:], id_d[:])
            # eye8 replicated across partitions: eyeRep[p, n, col] = (col==n)
            eye_rep = constp.tile([P, NB, NB], bf16)
            nc.sync.dma_start(eye_rep[:], eye_d[:].to_broadcast((P, NB, NB)))
            bias_sb = constp.tile([NB, 1], f32)
            nc.vector.memset(bias_sb[:], -20.0)
            mask_sb = bigp.tile([NB, T], f32)
            nc.sync.dma_start(mask_sb[:], m_d[:])

            p_sb = bigp.tile([NB, T], f32)
            parts = bigp.tile([NB, GROUPS], f32)
            pT_sb = bigp.tile([P, NCHUNKS, NB], bf16)
            ctx_ps = pscp.tile([NB, D], f32)

            e_tiles = {}

            # one big DMA per batch per tensor — issued upfront so the DMA
            # queues stream at line rate with no slot/issue stalls
            kt_tiles, v_tiles = [], []
            for n in range(NB):
                kt = kvp.tile([P, DH, T], f16, tag="kt", bufs=8)
                nc.sync.dma_start(kt[:], kt_d[n])
                kt_tiles.append(kt)
                vt = kvp.tile([P, NCHUNKS, D], bf16, tag="vt", bufs=8)
                nc.sync.dma_start(vt[:], v_d[n])
                v_tiles.append(vt)

            def emit_energy(g):
                e_ps = psep.tile([NB, GT], f32, tag="eps")
                e_tiles[g] = e_ps
                for n in range(NB):
                    for dh in range(DH):
                        nc.tensor.matmul(
                            e_ps[:],
                            qt_sb[:, dh, n, :],
                            kt_tiles[n][:, dh, g * GT : (g + 1) * GT],
                            start=(n == 0 and dh == 0),
                            stop=(n == NB - 1 and dh == DH - 1),
                        )

            def emit_posts(g):
                sl = slice(g * GT, (g + 1) * GT)
                em = smp.tile([NB, GT], f32, tag="em")
                nc.vector.tensor_tensor(
                    em[:], e_tiles[g][:], mask_sb[:, sl], Alu.add
                )
                nc.scalar.activation(
                    p_sb[:, sl],
                    em[:],
                    mybir.ActivationFunctionType.Exp,
                    bias=bias_sb[:],
                    accum_out=parts[:, g : g + 1],
                )
                for c in range(GC):
                    cg = g * GC + c
                    pT_ps = pstp.tile([P, NB], f32, tag="pt")
                    nc.tensor.transpose(
                        pT_ps[:], p_sb[:, cg * P : (cg + 1) * P], ident[:]
                    )
                    nc.vector.tensor_copy(pT_sb[:, cg, :], pT_ps[:])
                for n in range(NB):
                    vt = v_tiles[n]
                    for c in range(GC):
                        cg = g * GC + c
                        # masked slab: only column n of p survives, so the
                        # matmul writes ctx row n and zeros elsewhere
                        pTm = smp.tile([P, NB], bf16, tag="ptm", bufs=16)
                        nc.vector.tensor_tensor(
                            pTm[:], pT_sb[:, cg, :], eye_rep[:, n, :], Alu.mult
                        )
                        nc.tensor.matmul(
                            ctx_ps[:],
                            pTm[:],
                            vt[:, cg, :],
                            start=(g == 0 and n == 0 and c == 0),
                            stop=(
                                g == GROUPS - 1 and n == NB - 1 and c == GC - 1
                            ),
                        )

            for g in range(GROUPS):
                emit_energy(g)
                if g >= 1:
                    emit_posts(g - 1)
            emit_posts(GROUPS - 1)

            # tail: s, r, normalize both outputs
            s_sb = smp.tile([NB, 1], f32, tag="s")
            nc.vector.tensor_reduce(s_sb[:], parts[:], mybir.AxisListType.X, Alu.add)
            r_sb = smp.tile([NB, 1], f32, tag="r")
            nc.vector.reciprocal(r_sb[:], s_sb[:])
            att_sb = bigp.tile([NB, T], f32)
            nc.vector.tensor_scalar_mul(att_sb[:], p_sb[:], r_sb[:])
            nc.scalar.dma_start(att_d[:], att_sb[:])
            ctx_sb = smp.tile([NB, D], f32, tag="ctx")
            nc.scalar.mul(ctx_sb[:], ctx_ps[:], mul=r_sb[:])
            nc.scalar.dma_start(ctx_d[:], ctx_sb[:])

    return nc


def _split_multiwaits(bir):
    """The walrus build in this container allows only ONE sync wait per
    instruction (setupSyncWait: 'Too many sync wait commands'). Tile attaches
    multiple waits to single instructions. Split the extras into standalone
    EventSemaphore wait instructions (same engine, placed immediately before)
    — semantically identical, just sequential waits."""
    ctr = 0
    for fn in bir["functions"]:
        for blk in fn["blocks"]:
            out = []
            for inst in blk["instructions"]:
                si = inst.get("sync_info")
                waits = (si or {}).get("on_wait") or []
                # raw-bytes ISA instructions encode waits in their payload;
                # rewriting sync_info desyncs the encoded length
                if len(waits) > 1 and not inst.get("instr"):
                    for w in waits[:-1]:
                        ctr += 1
                        pre = {
                            "name": f"I-mw{ctr}",
                            "opcode": "EventSemaphore",
                            "engine": inst["engine"],
                            "ins": [],
                            "outs": [],
                            "sync_info": {"on_update": [], "on_wait": [w]},
                        }
                        if "debug" in inst:
                            pre["debug"] = inst["debug"]
                        out.append(pre)
                    si["on_wait"] = [waits[-1]]
                out.append(inst)
            blk["instructions"] = out
    return bir


def _patch_json(nc):
    import json as _json

    orig = nc.to_json_bytes

    def patched():
        bir = _json.loads(orig())
        _split_multiwaits(bir)
        return _json.dumps(bir).encode()

    nc.to_json_bytes = patched
    return nc


def _get_nc():
    if "nc" not in _CACHE:
        _CACHE["nc"] = _patch_json(_build())
    return _CACHE["nc"]


def _make_in_maps(query, key, value, lens):
    import ml_dtypes

    query = np.asarray(query, dtype=np.float32)
    key = np.asarray(key, dtype=np.float32)
    value = np.asarray(value, dtype=np.float32)
    lens = np.asarray(lens).astype(np.int64)

    addmask = np.where(
        np.arange(T)[None, :] < lens[:, None], 0.0, -1e9
    ).astype(np.float32)
    ident8 = np.eye(NB, dtype=np.float32)
    eye8bf = np.eye(NB, dtype=ml_dtypes.bfloat16)[None]

    in_maps = []
    for i in range(NCORES):
        sl = slice(i * NB, (i + 1) * NB)
        q = query[sl]  # (NB, D)
        # qT[p, dh, n] = q[n, dh*128+p]; masked slab qTm[..., n, col!=n] = 0
        qT = q.T.reshape(DH, P, NB).transpose(1, 0, 2)
        qTm = np.zeros((P, DH, NB, NB), dtype=np.float16)
        qTm[:, :, np.arange(NB), np.arange(NB)] = qT.astype(np.float16)
        keyT = np.ascontiguousarray(
            key[sl]
            .transpose(0, 2, 1)
            .reshape(NB, DH, P, T)
            .transpose(0, 2, 1, 3)
            .astype(np.float16)
        )
        valP = np.ascontiguousarray(
            value[sl]
            .reshape(NB, GROUPS, GC, P, D)
            .transpose(0, 3, 1, 2, 4)
            .reshape(NB, P, NCHUNKS, D)
            .astype(ml_dtypes.bfloat16)
        )
        in_maps.append(
            {
                "qTm": qTm,
                "keyT": keyT,
                "valP": valP,
                "addmask": np.ascontiguousarray(addmask[sl]),
                "ident8": ident8,
                "eye8bf": eye8bf,
            }
        )
    return in_maps


def _run(in_maps, trace=False, **kwargs):
    from concourse.bass_utils import run_bass_kernel_spmd

    nc = _get_nc()
    return run_bass_kernel_spmd(
        nc, in_maps, core_ids=list(range(NCORES)), trace=trace, **kwargs
    )


def kernel(query, key, value, lens):
    in_maps = _make_in_maps(query, key, value, lens)
    res = _run(in_maps, trace=False)
    ctx = np.concatenate([r["ctx_out"] for r in res.results], axis=0)
    att = np.concatenate([r["att_out"] for r in res.results], axis=0)
    return ctx, att


# revision 43
# speedup vs baseline: 1.5962x; 1.1979x over previous
"""Ragged-sequence attention kernel for 8 Trainium2 NeuronCores.

Problem: N=64 batches, T=2048, D=256.
  energy[n,t] = <key[n,t,:], query[n,:]>
  att = softmax(energy) masked to t < lens[n], renormalized
  context[n,:] = sum_t att[n,t] * value[n,t,:]

Math identity used: global softmax + mask + renormalize == masked softmax
(the global denominator cancels), so we compute p = exp(e - 20 + addmask)
with addmask in {0, -1e9}, accumulate UNNORMALIZED context = sum_t p*v, and
apply r = 1/sum_t(p) to both outputs at the very end.

Sharding: pure data parallel, batch dim split 8 ways (8 batches/core).

Device structure (v8):
  - energy on PE: host-transposed keyT[n] = key[n].T so e = qT.T @ keyT runs
    as N=512 matmuls; energy lands [8 batches x T] with batches in
    partitions, making the whole softmax a handful of 8-lane ops.
  - context on PE in bf16 (value uploaded pre-cast + pre-permuted), with
    per-128-chunk PE transposes of p to put t into partitions.
  - t processed in 4 groups of 512, software-pipelined: group g's energy
    matmuls run on PE while group g-1's softmax (DVE/ACT) completes, so PE
    never waits on the softmax chain.
"""

import numpy as np

N, T, D = 64, 2048, 256
NCORES = 8
NB = N // NCORES          # 8 batches per core
P = 128
GROUPS = 4
GT = T // GROUPS          # 512 timesteps per group
GC = GT // P              # 4 chunks of 128 t per group
NCHUNKS = T // P          # 16 chunks total
DH = D // P               # 2 halves of the d contraction

_CACHE = {}


def _build():
    import concourse.bass as bass
    import concourse.tile as tile
    from concourse import mybir

    f32 = mybir.dt.float32
    bf16 = mybir.dt.bfloat16
    f16 = mybir.dt.float16
    Alu = mybir.AluOpType

    nc = bass.Bass()

    # qTm[p, dh, n, col] = query[n, dh*128+p] if col==n else 0 — masked weight
    # slab so each energy matmul writes the full [NB, GT] PSUM tile (PE output
    # base partition must be 0/32/64, so per-row outputs are not allowed)
    qt_d = nc.declare_dram_parameter("qTm", [P, DH, NB, NB], f16, isOutput=False)
    # keyT[n] = key[n].T  (host-transposed, f32)
    # kth[n, p, dh, t] = key[n, t, dh*128+p] — 8 KB contiguous per partition
    kt_d = nc.declare_dram_parameter("keyT", [NB, P, DH, T], f16, isOutput=False)
    # valP[n, g, p, c, d] = value[n, g*512 + c*128 + p, d]  (bf16)
    # valP[n, p, cg, d] = value[n, (cg//4)*512 + (cg%4)*128 + p, d]
    v_d = nc.declare_dram_parameter("valP", [NB, P, NCHUNKS, D], bf16, isOutput=False)
    m_d = nc.declare_dram_parameter("addmask", [NB, T], f32, isOutput=False)
    id_d = nc.declare_dram_parameter("ident8", [NB, NB], f32, isOutput=False)
    eye_d = nc.declare_dram_parameter("eye8bf", [1, NB, NB], bf16, isOutput=False)
    ctx_d = nc.declare_dram_parameter("ctx_out", [NB, D], f32, isOutput=True)
    att_d = nc.declare_dram_parameter("att_out", [NB, T], f32, isOutput=True)


    with tile.TileContext(nc) as tc:
        with (
            tc.tile_pool(name="const", bufs=1) as constp,
            tc.tile_pool(name="kv", bufs=1) as kvp,
            tc.tile_pool(name="big", bufs=1) as bigp,
            tc.tile_pool(name="small", bufs=2) as smp,
            tc.tile_pool(name="pst", bufs=2, space="PSUM") as pstp,
            tc.tile_pool(name="pse", bufs=2, space="PSUM") as psep,
            tc.tile_pool(name="psc", bufs=1, space="PSUM") as pscp,
        ):
            qt_sb = constp.tile([P, DH, NB, NB], f16)
            nc.sync.dma_start(qt_sb[:], qt_d[:])
            ident = constp.tile([NB, NB], f32)
            nc.sync.dma_start(ident[:], id_d[:])
            # eye8 replicated across partitions: eyeRep[p, n, col] = (col==n)
            eye_rep = constp.tile([P, NB, NB], bf16)
            nc.sync.dma_start(eye_rep[:], eye_d[:].to_broadcast((P, NB, NB)))
            bias_sb = constp.tile([NB, 1], f32)
            nc.vector.memset(bias_sb[:], -20.0)
            mask_sb = bigp.tile([NB, T], f32)
            nc.sync.dma_start(mask_sb[:], m_d[:])

            p_sb = bigp.tile([NB, T], f32)
            parts = bigp.tile([NB, GROUPS], f32)
            pT_sb = bigp.tile([P, NCHUNKS, NB], bf16)
            ctx_ps = pscp.tile([NB, D], f32)

            e_tiles = {}

            # separate tile per (batch, group), all DMAs issued upfront in
            # group order: group 0's 4.2 MiB lands first so PE starts early
            # and pipelines with the remaining stream
            kt_tiles, v_tiles = {}, {}
            for g in range(GROUPS):
                for n in range(NB):
                    kt = kvp.tile([P, DH, GT], f16, tag="kt", bufs=32)
                    nc.sync.dma_start(
                        kt[:], kt_d[n][:, :, g * GT : (g + 1) * GT]
                    )
                    kt_tiles[(n, g)] = kt
                    vt = kvp.tile([P, GC, D], bf16, tag="vt", bufs=32)
                    nc.sync.dma_start(
                        vt[:], v_d[n][:, g * GC : (g + 1) * GC, :]
                    )
                    v_tiles[(n, g)] = vt

            def emit_energy(g):
                e_ps = psep.tile([NB, GT], f32, tag="eps")
                e_tiles[g] = e_ps
                for n in range(NB):
                    for dh in range(DH):
                        nc.tensor.matmul(
                            e_ps[:],
                            qt_sb[:, dh, n, :],
                            kt_tiles[(n, g)][:, dh, :],
                            start=(n == 0 and dh == 0),
                            stop=(n == NB - 1 and dh == DH - 1),
                        )

            def emit_posts(g):
                sl = slice(g * GT, (g + 1) * GT)
                em = smp.tile([NB, GT], f32, tag="em")
                nc.vector.tensor_tensor(
                    em[:], e_tiles[g][:], mask_sb[:, sl], Alu.add
                )
                nc.scalar.activation(
                    p_sb[:, sl],
                    em[:],
                    mybir.ActivationFunctionType.Exp,
                    bias=bias_sb[:],
                    accum_out=parts[:, g : g + 1],
                )
                for c in range(GC):
                    cg = g * GC + c
                    pT_ps = pstp.tile([P, NB], f32, tag="pt")
                    nc.tensor.transpose(
                        pT_ps[:], p_sb[:, cg * P : (cg + 1) * P], ident[:]
                    )
                    nc.vector.tensor_copy(pT_sb[:, cg, :], pT_ps[:])
                for n in range(NB):
                    vt = v_tiles[(n, g)]
                    for c in range(GC):
                        cg = g * GC + c
                        # masked slab: only column n of p survives, so the
                        # matmul writes ctx row n and zeros elsewhere
                        pTm = smp.tile([P, NB], bf16, tag="ptm", bufs=16)
                        nc.vector.tensor_tensor(
                            pTm[:], pT_sb[:, cg, :], eye_rep[:, n, :], Alu.mult
                        )
                        nc.tensor.matmul(
                            ctx_ps[:],
                            pTm[:],
                            vt[:, c, :],
                            start=(g == 0 and n == 0 and c == 0),
                            stop=(
                                g == GROUPS - 1 and n == NB - 1 and c == GC - 1
                            ),
                        )

            for g in range(GROUPS):
                emit_energy(g)
                if g >= 1:
                    emit_posts(g - 1)
            emit_posts(GROUPS - 1)

            # tail: s, r, normalize both outputs
            s_sb = smp.tile([NB, 1], f32, tag="s")
            nc.vector.tensor_reduce(s_sb[:], parts[:], mybir.AxisListType.X, Alu.add)
            r_sb = smp.tile([NB, 1], f32, tag="r")
            nc.vector.reciprocal(r_sb[:], s_sb[:])
            att_sb = bigp.tile([NB, T], f32)
            nc.vector.tensor_scalar_mul(att_sb[:], p_sb[:], r_sb[:])
            nc.scalar.dma_start(att_d[:], att_sb[:])
            ctx_sb = smp.tile([NB, D], f32, tag="ctx")
            nc.scalar.mul(ctx_sb[:], ctx_ps[:], mul=r_sb[:])
            nc.scalar.dma_start(ctx_d[:], ctx_sb[:])

    return nc


def _split_multiwaits(bir):
    """The walrus build in this container allows only ONE sync wait per
    instruction (setupSyncWait: 'Too many sync wait commands'). Tile attaches
    multiple waits to single instructions. Split the extras into standalone
    EventSemaphore wait instructions (same engine, placed immediately before)
    — semantically identical, just sequential waits."""
    ctr = 0
    for fn in bir["functions"]:
        for blk in fn["blocks"]:
            out = []
            for inst in blk["instructions"]:
                si = inst.get("sync_info")
                waits = (si or {}).get("on_wait") or []
                # raw-bytes ISA instructions encode waits in their payload;
                # rewriting sync_info desyncs the encoded length
                if len(waits) > 1 and not inst.get("instr"):
                    for w in waits[:-1]:
                        ctr += 1
                        pre = {
                            "name": f"I-mw{ctr}",
                            "opcode": "EventSemaphore",
                            "engine": inst["engine"],
                            "ins": [],
                            "outs": [],
                            "sync_info": {"on_update": [], "on_wait": [w]},
                        }
                        if "debug" in inst:
                            pre["debug"] = inst["debug"]
                        out.append(pre)
                    si["on_wait"] = [waits[-1]]
                out.append(inst)
            blk["instructions"] = out
    return bir


def _patch_json(nc):
    import json as _json

    orig = nc.to_json_bytes

    def patched():
        bir = _json.loads(orig())
        _split_multiwaits(bir)
        return _json.dumps(bir).encode()

    nc.to_json_bytes = patched
    return nc


def _get_nc():
    if "nc" not in _CACHE:
        _CACHE["nc"] = _patch_json(_build())
    return _CACHE["nc"]


def _make_in_maps(query, key, value, lens):
    import ml_dtypes

    query = np.asarray(query, dtype=np.float32)
    key = np.asarray(key, dtype=np.float32)
    value = np.asarray(value, dtype=np.float32)
    lens = np.asarray(lens).astype(np.int64)

    addmask = np.where(
        np.arange(T)[None, :] < lens[:, None], 0.0, -1e9
    ).astype(np.float32)
    ident8 = np.eye(NB, dtype=np.float32)
    eye8bf = np.eye(NB, dtype=ml_dtypes.bfloat16)[None]

    in_maps = []
    for i in range(NCORES):
        sl = slice(i * NB, (i + 1) * NB)
        q = query[sl]  # (NB, D)
        # qT[p, dh, n] = q[n, dh*128+p]; masked slab qTm[..., n, col!=n] = 0
        qT = q.T.reshape(DH, P, NB).transpose(1, 0, 2)
        qTm = np.zeros((P, DH, NB, NB), dtype=np.float16)
        qTm[:, :, np.arange(NB), np.arange(NB)] = qT.astype(np.float16)
        keyT = np.ascontiguousarray(
            key[sl]
            .transpose(0, 2, 1)
            .reshape(NB, DH, P, T)
            .transpose(0, 2, 1, 3)
            .astype(np.float16)
        )
        valP = np.ascontiguousarray(
            value[sl]
            .reshape(NB, GROUPS, GC, P, D)
            .transpose(0, 3, 1, 2, 4)
            .reshape(NB, P, NCHUNKS, D)
            .astype(ml_dtypes.bfloat16)
        )
        in_maps.append(
            {
                "qTm": qTm,
                "keyT": keyT,
                "valP": valP,
                "addmask": np.ascontiguousarray(addmask[sl]),
                "ident8": ident8,
                "eye8bf": eye8bf,
            }
        )
    return in_maps


def _run(in_maps, trace=False, **kwargs):
    from concourse.bass_utils import run_bass_kernel_spmd

    nc = _get_nc()
    return run_bass_kernel_spmd(
        nc, in_maps, core_ids=list(range(NCORES)), trace=trace, **kwargs
    )


def kernel(query, key, value, lens):
    in_maps = _make_in_maps(query, key, value, lens)
    res = _run(in_maps, trace=False)
    ctx = np.concatenate([r["ctx_out"] for r in res.results], axis=0)
    att = np.concatenate([r["att_out"] for r in res.results], axis=0)
    return ctx, att
